# revision 1
# baseline (speedup 1.0000x reference)
"""MoE layer (top-k routing) on 8 Trainium2 NeuronCores.

Strategy (expert-parallel, per sharding hint):
  - Host: compute router softmax + top-k (0.1% of FLOPs), build per-expert
    token batches (the "all-to-all dispatch by expert assignment" is realized
    while constructing the per-core SPMD input maps).
  - Core e owns expert e: computes y = (gelu(x @ w1.T + b1) @ w2.T + b2) * w
    for its gathered tokens in bf16 (fp32 PSUM accumulation), with the
    combine weight multiplied in on-device.
  - Host: scatter-add the per-expert results back into the [B,N,C] output.

Data layout on device is "transposed activation" space so both GEMMs chain
without transposes: xT [C, T], hT [F, T], yT [C, T] with the contraction dim
on SBUF partitions. Weights are passed pre-transposed (w1T = w1.T [C,F],
w2T = w2.T [F,C]) and stay SBUF-resident for the whole kernel.
"""

import json
import os
import sys
import types

import numpy as np
import ml_dtypes

D_MODEL = 1024
D_FF = 4096
N_EXPERTS = 8
N_CORES = 8
B, N = 8, 2048
T = B * N

P = 128
CB = D_MODEL // P      # 8 c-blocks of 128
FB = D_FF // P         # 32 f-blocks of 128
TN = 512               # token tile (matmul moving free dim / one PSUM bank)


def _shim_axon_hooks():
    """Register the NTFF profile hook bass_utils looks for under axon.

    The image's `antenv` stub lacks `axon_hooks`; the ctypes machinery in
    trn_agent_boot provides the same hook. Only needed for trace runs, but
    harmless otherwise."""
    if "antenv.axon_hooks" in sys.modules:
        return
    try:
        import trn_agent_boot.trn_boot as _tb
        hook = _tb._ntff_profile_via_ctypes("/opt/axon/libaxon_pjrt.so")
    except Exception:
        hook = None
    mod = types.ModuleType("antenv.axon_hooks")
    mod.get_axon_ntff_profile_hook = lambda: hook
    mod.set_axon_ntff_profile_hook = lambda h: None
    sys.modules["antenv.axon_hooks"] = mod


_shim_axon_hooks()

import concourse.bass as bass            # noqa: E402
import concourse.tile as tile            # noqa: E402
from concourse import mybir              # noqa: E402
from concourse.bass import ds, ts        # noqa: E402
from concourse.bass_utils import run_bass_kernel_spmd  # noqa: E402


def _fix_multiwait_bir(nc):
    """Split instructions carrying >1 sync wait (the TileContext tail drain)
    into single-wait NoOps + the original instruction with one wait; this
    walrus build rejects multi-wait CTRL instructions."""
    raw = bass.Bass.to_json_bytes(nc)
    d = json.loads(raw)
    for f in d["functions"]:
        for b in f["blocks"]:
            out = []
            for i in b["instructions"]:
                si = i.get("sync_info") or {}
                waits = si.get("on_wait") or []
                if len(waits) > 1:
                    for k, w in enumerate(waits[:-1]):
                        out.append({
                            "name": f"{i['name']}_wsplit{k}",
                            "engine": i["engine"],
                            "ins": [], "outs": [],
                            "opcode": "NoOp",
                            "sync_info": {"on_update": [], "on_wait": [w]},
                        })
                    si["on_wait"] = [waits[-1]]
                out.append(i)
            b["instructions"] = out
    fixed = json.dumps(d).encode()
    nc.to_json_bytes = lambda: fixed


_NC_CACHE = {}


def _build_moe_kernel(cap):
    """One expert's FFN over `cap` gathered tokens (SPMD on all 8 cores)."""
    if cap in _NC_CACHE:
        return _NC_CACHE[cap]

    bf16 = mybir.dt.bfloat16
    f32 = mybir.dt.float32
    Act = mybir.ActivationFunctionType

    nc = bass.Bass("TRN2", target_bir_lowering=False, debug=False,
                   num_devices=N_CORES)
    xT = nc.declare_dram_parameter("xT", [D_MODEL, cap], bf16, isOutput=False)
    w1t = nc.declare_dram_parameter("w1t", [D_MODEL, D_FF], bf16, isOutput=False)
    w2t = nc.declare_dram_parameter("w2t", [D_FF, D_MODEL], bf16, isOutput=False)
    b1 = nc.declare_dram_parameter("b1", [D_FF], f32, isOutput=False)
    b2 = nc.declare_dram_parameter("b2", [D_MODEL], f32, isOutput=False)
    wts = nc.declare_dram_parameter("wts", [P, cap], f32, isOutput=False)
    yT = nc.declare_dram_parameter("yT", [D_MODEL, cap], f32, isOutput=True)

    # DRAM views with 128-partition blocks pulled out
    xr = xT.ap().rearrange("(g p) t -> p g t", p=P)     # [128, 8, cap]
    w1r = w1t.ap().rearrange("(g p) f -> p g f", p=P)   # [128, 8, 4096]
    w2r = w2t.ap().rearrange("(g p) c -> p g c", p=P)   # [128, 32, 1024]
    b1r = b1.ap().rearrange("(g p) -> p g", p=P)        # [128, 32]
    b2r = b2.ap().rearrange("(g p) -> p g", p=P)        # [128, 8]
    yr = yT.ap().rearrange("(g p) t -> p g t", p=P)     # [128, 8, cap]

    # token tiles: full 512s plus an optional 128-multiple remainder
    tiles = []
    off = 0
    while off < cap:
        tw = min(TN, cap - off)
        tiles.append((off, tw))
        off += tw

    with tile.TileContext(nc) as tc:
        with (
            tc.tile_pool(name="weights", bufs=1) as wpool,
            tc.tile_pool(name="xin", bufs=3) as xpool,
            tc.tile_pool(name="wtp", bufs=3) as wtpool,
            tc.tile_pool(name="hbuf", bufs=1) as hpool,
            tc.tile_pool(name="yout", bufs=4) as ypool,
            tc.tile_pool(name="psum", bufs=4, space="PSUM") as psum,
        ):
            b1_sb = wpool.tile([P, FB], f32, tag="b1")
            nc.sync.dma_start(b1_sb[:], b1r)
            b2_sb = wpool.tile([P, CB], f32, tag="b2")
            nc.sync.dma_start(b2_sb[:], b2r)

            w1_sb = wpool.tile([P, CB, D_FF], bf16, tag="w1")
            for k in range(CB):
                nc.sync.dma_start(w1_sb[:, k, :], w1r[:, k, :])
            w2_sb = wpool.tile([P, FB, D_MODEL], bf16, tag="w2")
            for k in range(0, FB, 4):
                nc.sync.dma_start(w2_sb[:, k:k + 4, :], w2r[:, k:k + 4, :])

            for (off, tw) in tiles:
                xt = xpool.tile([P, CB, TN], bf16, tag="xt")
                nc.sync.dma_start(xt[:, :, :tw], xr[:, :, ds(off, tw)])
                wt = wtpool.tile([P, TN], f32, tag="wt")
                nc.sync.dma_start(wt[:, :tw], wts[:, ds(off, tw)])

                ht = hpool.tile([P, FB, TN], bf16, tag="ht")
                # GEMM1 + gelu: hT[f,t] = gelu(sum_c w1T[c,f] * xT[c,t] + b1[f])
                for m in range(FB):
                    ph = psum.tile([P, TN], f32, tag="ph")
                    for k in range(CB):
                        nc.tensor.matmul(
                            ph[:, :tw],
                            lhsT=w1_sb[:, k, ts(m, P)],
                            rhs=xt[:, k, :tw],
                            start=(k == 0), stop=(k == CB - 1),
                        )
                    nc.scalar.activation(ht[:, m, :tw], ph[:, :tw], Act.Gelu,
                                         bias=b1_sb[:, m:m + 1])
                # GEMM2 + bias + combine-weight: yT[c,t]
                for c in range(CB):
                    py = psum.tile([P, TN], f32, tag="py")
                    for k in range(FB):
                        nc.tensor.matmul(
                            py[:, :tw],
                            lhsT=w2_sb[:, k, ts(c, P)],
                            rhs=ht[:, k, :tw],
                            start=(k == 0), stop=(k == FB - 1),
                        )
                    yt = ypool.tile([P, TN], f32, tag="yt")
                    nc.scalar.add(yt[:, :tw], py[:, :tw], b2_sb[:, c:c + 1])
                    nc.vector.tensor_mul(yt[:, :tw], yt[:, :tw], wt[:, :tw])
                    nc.sync.dma_start(yr[:, c, ds(off, tw)], yt[:, :tw])

    _fix_multiwait_bir(nc)
    _NC_CACHE[cap] = nc
    return nc


def _route(xf, router_w, k):
    """Replicate the reference router numerics (f32 softmax, top-k, renorm)."""
    logits = xf @ router_w.T.astype(np.float32)          # [T, E]
    m = logits.max(axis=-1, keepdims=True)
    e = np.exp(logits - m, dtype=np.float32)
    probs = e / e.sum(axis=-1, keepdims=True)
    # descending, ties -> lower index first (matches jax.lax.top_k)
    idx = np.argsort(-probs, axis=-1, kind="stable")[:, :k]   # [T, k]
    w = np.take_along_axis(probs, idx, axis=-1)               # [T, k]
    w = w / (w.sum(axis=-1, keepdims=True) + 1e-9)
    return idx, w


def kernel(x, router_w, expert_w1, expert_b1, expert_w2, expert_b2, top_k):
    x = np.asarray(x)
    router_w = np.asarray(router_w, dtype=np.float32)
    expert_w1 = np.asarray(expert_w1, dtype=np.float32)
    expert_b1 = np.asarray(expert_b1, dtype=np.float32)
    expert_w2 = np.asarray(expert_w2, dtype=np.float32)
    expert_b2 = np.asarray(expert_b2, dtype=np.float32)
    k = int(np.asarray(top_k))
    Bq, Nq, C = x.shape
    Tq = Bq * Nq
    E = expert_w1.shape[0]
    xf = np.ascontiguousarray(x.reshape(Tq, C), dtype=np.float32)

    idx, w = _route(xf, router_w, k)

    # per-expert token lists + combine weights
    tok_idx, tok_w = [], []
    for e in range(E):
        mask = idx == e                                   # [T, k]
        sel = np.nonzero(mask.any(axis=-1))[0]
        tok_idx.append(sel)
        tok_w.append((w * mask).sum(axis=-1)[sel].astype(np.float32))
    counts = [len(s) for s in tok_idx]
    cap = max(P, -(-max(counts) // P) * P)

    nc = _build_moe_kernel(cap)

    in_maps = []
    for e in range(E):
        cnt = counts[e]
        xT = np.zeros((C, cap), dtype=ml_dtypes.bfloat16)
        xT[:, :cnt] = xf[tok_idx[e]].T
        wtsP = np.zeros((P, cap), dtype=np.float32)
        wtsP[:, :cnt] = tok_w[e][None, :]
        in_maps.append({
            "xT": xT,
            "w1t": np.ascontiguousarray(expert_w1[e].T).astype(ml_dtypes.bfloat16),
            "w2t": np.ascontiguousarray(expert_w2[e].T).astype(ml_dtypes.bfloat16),
            "b1": np.ascontiguousarray(expert_b1[e]),
            "b2": np.ascontiguousarray(expert_b2[e]),
            "wts": wtsP,
        })

    trace = os.environ.get("BASS_MOE_TRACE") == "1"
    res = run_bass_kernel_spmd(
        nc, in_maps, core_ids=list(range(N_CORES)),
        trace=trace,
        tmpdir=os.environ.get("BASS_MOE_TMPDIR") if trace else None,
    )
    if trace:
        kernel.last_exec_time_ns = res.exec_time_ns
        kernel.last_trace = (res.instructions_and_trace or (None, None))[1]

    out = np.zeros((Tq, C), dtype=np.float32)
    for e in range(E):
        cnt = counts[e]
        if cnt:
            out[tok_idx[e]] += res.results[e]["yT"][:, :cnt].T
    return out.reshape(Bq, Nq, C).astype(x.dtype)


# revision 2
# speedup vs baseline: 1.0399x; 1.0399x over previous
"""MoE layer (top-k routing) on 8 Trainium2 NeuronCores.

Strategy (expert-parallel, per sharding hint):
  - Host: compute router softmax + top-k (0.1% of FLOPs), build per-expert
    token batches (the "all-to-all dispatch by expert assignment" is realized
    while constructing the per-core SPMD input maps).
  - Core e owns expert e: computes y = (gelu(x @ w1.T + b1) @ w2.T + b2) * w
    for its gathered tokens in bf16 (fp32 PSUM accumulation), with the
    combine weight multiplied in on-device.
  - Host: scatter-add the per-expert results back into the [B,N,C] output.

Data layout on device is "transposed activation" space so both GEMMs chain
without transposes: xT [C, T], hT [F, T], yT [C, T] with the contraction dim
on SBUF partitions. Weights are passed pre-transposed (w1T = w1.T [C,F],
w2T = w2.T [F,C]) and stay SBUF-resident for the whole kernel.
"""

import json
import os
import sys
import types

import numpy as np
import ml_dtypes

D_MODEL = 1024
D_FF = 4096
N_EXPERTS = 8
N_CORES = 8
B, N = 8, 2048
T = B * N

P = 128
CB = D_MODEL // P      # 8 c-blocks of 128
FB = D_FF // P         # 32 f-blocks of 128
TN = 512               # token tile (matmul moving free dim / one PSUM bank)


def _shim_axon_hooks():
    """Register the NTFF profile hook bass_utils looks for under axon.

    The image's `antenv` stub lacks `axon_hooks`; the ctypes machinery in
    trn_agent_boot provides the same hook. Only needed for trace runs, but
    harmless otherwise."""
    if "antenv.axon_hooks" in sys.modules:
        return
    try:
        import trn_agent_boot.trn_boot as _tb
        hook = _tb._ntff_profile_via_ctypes("/opt/axon/libaxon_pjrt.so")
    except Exception:
        hook = None
    mod = types.ModuleType("antenv.axon_hooks")
    mod.get_axon_ntff_profile_hook = lambda: hook
    mod.set_axon_ntff_profile_hook = lambda h: None
    sys.modules["antenv.axon_hooks"] = mod


_shim_axon_hooks()

import concourse.bass as bass            # noqa: E402
import concourse.tile as tile            # noqa: E402
from concourse import mybir              # noqa: E402
from concourse.bass import ds, ts        # noqa: E402
from concourse.bass_utils import run_bass_kernel_spmd  # noqa: E402


def _fix_multiwait_bir(nc):
    """Split instructions carrying >1 sync wait (the TileContext tail drain)
    into single-wait NoOps + the original instruction with one wait; this
    walrus build rejects multi-wait CTRL instructions."""
    raw = bass.Bass.to_json_bytes(nc)
    d = json.loads(raw)
    for f in d["functions"]:
        for b in f["blocks"]:
            out = []
            for i in b["instructions"]:
                si = i.get("sync_info") or {}
                waits = si.get("on_wait") or []
                if len(waits) > 1:
                    for k, w in enumerate(waits[:-1]):
                        out.append({
                            "name": f"{i['name']}_wsplit{k}",
                            "engine": i["engine"],
                            "ins": [], "outs": [],
                            "opcode": "NoOp",
                            "sync_info": {"on_update": [], "on_wait": [w]},
                        })
                    si["on_wait"] = [waits[-1]]
                out.append(i)
            b["instructions"] = out
    fixed = json.dumps(d).encode()
    nc.to_json_bytes = lambda: fixed


_NC_CACHE = {}


def _build_moe_kernel(cap):
    """One expert's FFN over `cap` gathered tokens (SPMD on all 8 cores)."""
    if cap in _NC_CACHE:
        return _NC_CACHE[cap]

    bf16 = mybir.dt.bfloat16
    f32 = mybir.dt.float32
    Act = mybir.ActivationFunctionType

    nc = bass.Bass("TRN2", target_bir_lowering=False, debug=False,
                   num_devices=N_CORES)
    xT = nc.declare_dram_parameter("xT", [D_MODEL, cap], bf16, isOutput=False)
    w1t = nc.declare_dram_parameter("w1t", [D_MODEL, D_FF], bf16, isOutput=False)
    w2t = nc.declare_dram_parameter("w2t", [D_FF, D_MODEL], bf16, isOutput=False)
    b1 = nc.declare_dram_parameter("b1", [D_FF], f32, isOutput=False)
    b2 = nc.declare_dram_parameter("b2", [D_MODEL], f32, isOutput=False)
    wts = nc.declare_dram_parameter("wts", [P, cap], f32, isOutput=False)
    yT = nc.declare_dram_parameter("yT", [D_MODEL, cap], f32, isOutput=True)

    # DRAM views with 128-partition blocks pulled out
    xr = xT.ap().rearrange("(g p) t -> p g t", p=P)     # [128, 8, cap]
    w1r = w1t.ap().rearrange("(g p) f -> p g f", p=P)   # [128, 8, 4096]
    w2r = w2t.ap().rearrange("(g p) c -> p g c", p=P)   # [128, 32, 1024]
    b1r = b1.ap().rearrange("(g p) -> p g", p=P)        # [128, 32]
    b2r = b2.ap().rearrange("(g p) -> p g", p=P)        # [128, 8]
    yr = yT.ap().rearrange("(g p) t -> p g t", p=P)     # [128, 8, cap]

    # token tiles: full 512s plus an optional 128-multiple remainder
    tiles = []
    off = 0
    while off < cap:
        tw = min(TN, cap - off)
        tiles.append((off, tw))
        off += tw

    with tile.TileContext(nc) as tc:
        with (
            tc.tile_pool(name="weights", bufs=1) as wpool,
            tc.tile_pool(name="xin", bufs=3) as xpool,
            tc.tile_pool(name="wtp", bufs=3) as wtpool,
            tc.tile_pool(name="hbuf", bufs=1) as hpool,
            tc.tile_pool(name="yout", bufs=4) as ypool,
            tc.tile_pool(name="psum", bufs=4, space="PSUM") as psum,
        ):
            # DMA emission order is the SP ring's FIFO issue order; it is
            # chosen so PE can start ~8us in: tile-0 tokens first, then w1 in
            # M-strips (GEMM1 m-block 0 only needs strip 0 + xt0), then w2
            # (fully landed by the time GEMM2 of tile 0 starts at ~60us).
            # Later tiles' token loads queue behind the weights; they are
            # small and needed much later.
            b1_sb = wpool.tile([P, FB], f32, tag="b1")
            nc.sync.dma_start(b1_sb[:], b1r)
            b2_sb = wpool.tile([P, CB], f32, tag="b2")
            nc.sync.dma_start(b2_sb[:], b2r)

            x0 = xpool.tile([P, CB, TN], bf16, tag="xt")
            nc.sync.dma_start(x0[:, :, :tiles[0][1]], xr[:, :, ds(0, tiles[0][1])])
            w0 = wtpool.tile([P, TN], f32, tag="wt")
            nc.sync.dma_start(w0[:, :tiles[0][1]], wts[:, ds(0, tiles[0][1])])

            MS = 512  # w1 M-strip width (4 m-blocks, ~1 MiB per DMA)
            w1_sb = wpool.tile([P, CB, D_FF], bf16, tag="w1")
            for s in range(0, D_FF, MS):
                nc.sync.dma_start(w1_sb[:, :, s:s + MS], w1r[:, :, s:s + MS])
            w2_sb = wpool.tile([P, FB, D_MODEL], bf16, tag="w2")
            for k in range(0, FB, 4):
                nc.sync.dma_start(w2_sb[:, k:k + 4, :], w2r[:, k:k + 4, :])

            for ti, (off, tw) in enumerate(tiles):
                if ti == 0:
                    xt, wt = x0, w0
                else:
                    xt = xpool.tile([P, CB, TN], bf16, tag="xt")
                    nc.sync.dma_start(xt[:, :, :tw], xr[:, :, ds(off, tw)])
                    wt = wtpool.tile([P, TN], f32, tag="wt")
                    nc.sync.dma_start(wt[:, :tw], wts[:, ds(off, tw)])

                ht = hpool.tile([P, FB, TN], bf16, tag="ht")
                # GEMM1 + gelu: hT[f,t] = gelu(sum_c w1T[c,f] * xT[c,t] + b1[f])
                for m in range(FB):
                    ph = psum.tile([P, TN], f32, tag="ph")
                    for k in range(CB):
                        nc.tensor.matmul(
                            ph[:, :tw],
                            lhsT=w1_sb[:, k, ts(m, P)],
                            rhs=xt[:, k, :tw],
                            start=(k == 0), stop=(k == CB - 1),
                        )
                    nc.scalar.activation(ht[:, m, :tw], ph[:, :tw], Act.Gelu,
                                         bias=b1_sb[:, m:m + 1])
                # GEMM2 + bias + combine-weight: yT[c,t]
                for c in range(CB):
                    py = psum.tile([P, TN], f32, tag="py")
                    for k in range(FB):
                        nc.tensor.matmul(
                            py[:, :tw],
                            lhsT=w2_sb[:, k, ts(c, P)],
                            rhs=ht[:, k, :tw],
                            start=(k == 0), stop=(k == FB - 1),
                        )
                    yt = ypool.tile([P, TN], f32, tag="yt")
                    nc.scalar.add(yt[:, :tw], py[:, :tw], b2_sb[:, c:c + 1])
                    nc.vector.tensor_mul(yt[:, :tw], yt[:, :tw], wt[:, :tw])
                    nc.sync.dma_start(yr[:, c, ds(off, tw)], yt[:, :tw])

    _fix_multiwait_bir(nc)
    _NC_CACHE[cap] = nc
    return nc


def _route(xf, router_w, k):
    """Replicate the reference router numerics (f32 softmax, top-k, renorm)."""
    logits = xf @ router_w.T.astype(np.float32)          # [T, E]
    m = logits.max(axis=-1, keepdims=True)
    e = np.exp(logits - m, dtype=np.float32)
    probs = e / e.sum(axis=-1, keepdims=True)
    # descending, ties -> lower index first (matches jax.lax.top_k)
    idx = np.argsort(-probs, axis=-1, kind="stable")[:, :k]   # [T, k]
    w = np.take_along_axis(probs, idx, axis=-1)               # [T, k]
    w = w / (w.sum(axis=-1, keepdims=True) + 1e-9)
    return idx, w


def kernel(x, router_w, expert_w1, expert_b1, expert_w2, expert_b2, top_k):
    x = np.asarray(x)
    router_w = np.asarray(router_w, dtype=np.float32)
    expert_w1 = np.asarray(expert_w1, dtype=np.float32)
    expert_b1 = np.asarray(expert_b1, dtype=np.float32)
    expert_w2 = np.asarray(expert_w2, dtype=np.float32)
    expert_b2 = np.asarray(expert_b2, dtype=np.float32)
    k = int(np.asarray(top_k))
    Bq, Nq, C = x.shape
    Tq = Bq * Nq
    E = expert_w1.shape[0]
    xf = np.ascontiguousarray(x.reshape(Tq, C), dtype=np.float32)

    idx, w = _route(xf, router_w, k)

    # per-expert token lists + combine weights
    tok_idx, tok_w = [], []
    for e in range(E):
        mask = idx == e                                   # [T, k]
        sel = np.nonzero(mask.any(axis=-1))[0]
        tok_idx.append(sel)
        tok_w.append((w * mask).sum(axis=-1)[sel].astype(np.float32))
    counts = [len(s) for s in tok_idx]
    cap = max(P, -(-max(counts) // P) * P)

    nc = _build_moe_kernel(cap)

    in_maps = []
    for e in range(E):
        cnt = counts[e]
        xT = np.zeros((C, cap), dtype=ml_dtypes.bfloat16)
        xT[:, :cnt] = xf[tok_idx[e]].T
        wtsP = np.zeros((P, cap), dtype=np.float32)
        wtsP[:, :cnt] = tok_w[e][None, :]
        in_maps.append({
            "xT": xT,
            "w1t": np.ascontiguousarray(expert_w1[e].T).astype(ml_dtypes.bfloat16),
            "w2t": np.ascontiguousarray(expert_w2[e].T).astype(ml_dtypes.bfloat16),
            "b1": np.ascontiguousarray(expert_b1[e]),
            "b2": np.ascontiguousarray(expert_b2[e]),
            "wts": wtsP,
        })

    trace = os.environ.get("BASS_MOE_TRACE") == "1"
    res = run_bass_kernel_spmd(
        nc, in_maps, core_ids=list(range(N_CORES)),
        trace=trace,
        tmpdir=os.environ.get("BASS_MOE_TMPDIR") if trace else None,
    )
    if trace:
        kernel.last_exec_time_ns = res.exec_time_ns
        kernel.last_trace = (res.instructions_and_trace or (None, None))[1]

    out = np.zeros((Tq, C), dtype=np.float32)
    for e in range(E):
        cnt = counts[e]
        if cnt:
            out[tok_idx[e]] += res.results[e]["yT"][:, :cnt].T
    return out.reshape(Bq, Nq, C).astype(x.dtype)


# revision 3
# speedup vs baseline: 1.0440x; 1.0039x over previous
"""MoE layer (top-k routing) on 8 Trainium2 NeuronCores.

Strategy (expert-parallel, per sharding hint):
  - Host: compute router softmax + top-k (0.1% of FLOPs), build per-expert
    token batches (the "all-to-all dispatch by expert assignment" is realized
    while constructing the per-core SPMD input maps).
  - Core e owns expert e: computes y = (gelu(x @ w1.T + b1) @ w2.T + b2) * w
    for its gathered tokens in bf16 (fp32 PSUM accumulation), with the
    combine weight multiplied in on-device.
  - Host: scatter-add the per-expert results back into the [B,N,C] output.

Data layout on device is "transposed activation" space so both GEMMs chain
without transposes: xT [C, T], hT [F, T], yT [C, T] with the contraction dim
on SBUF partitions. Weights are passed pre-transposed (w1T = w1.T [C,F],
w2T = w2.T [F,C]) and stay SBUF-resident for the whole kernel.
"""

import json
import os
import sys
import types

import numpy as np
import ml_dtypes

D_MODEL = 1024
D_FF = 4096
N_EXPERTS = 8
N_CORES = 8
B, N = 8, 2048
T = B * N

P = 128
CB = D_MODEL // P      # 8 c-blocks of 128
FB = D_FF // P         # 32 f-blocks of 128
TN = 512               # token tile (matmul moving free dim / one PSUM bank)


def _shim_axon_hooks():
    """Register the NTFF profile hook bass_utils looks for under axon.

    The image's `antenv` stub lacks `axon_hooks`; the ctypes machinery in
    trn_agent_boot provides the same hook. Only needed for trace runs, but
    harmless otherwise."""
    if "antenv.axon_hooks" in sys.modules:
        return
    try:
        import trn_agent_boot.trn_boot as _tb
        hook = _tb._ntff_profile_via_ctypes("/opt/axon/libaxon_pjrt.so")
    except Exception:
        hook = None
    mod = types.ModuleType("antenv.axon_hooks")
    mod.get_axon_ntff_profile_hook = lambda: hook
    mod.set_axon_ntff_profile_hook = lambda h: None
    sys.modules["antenv.axon_hooks"] = mod


_shim_axon_hooks()

import concourse.bass as bass            # noqa: E402
import concourse.tile as tile            # noqa: E402
from concourse import mybir              # noqa: E402
from concourse.bass import ds, ts        # noqa: E402
from concourse.bass_utils import run_bass_kernel_spmd  # noqa: E402


def _fix_multiwait_bir(nc):
    """Split instructions carrying >1 sync wait (the TileContext tail drain)
    into single-wait NoOps + the original instruction with one wait; this
    walrus build rejects multi-wait CTRL instructions."""
    raw = bass.Bass.to_json_bytes(nc)
    d = json.loads(raw)
    for f in d["functions"]:
        for b in f["blocks"]:
            out = []
            for i in b["instructions"]:
                si = i.get("sync_info") or {}
                waits = si.get("on_wait") or []
                if len(waits) > 1:
                    for k, w in enumerate(waits[:-1]):
                        out.append({
                            "name": f"{i['name']}_wsplit{k}",
                            "engine": i["engine"],
                            "ins": [], "outs": [],
                            "opcode": "NoOp",
                            "sync_info": {"on_update": [], "on_wait": [w]},
                        })
                    si["on_wait"] = [waits[-1]]
                out.append(i)
            b["instructions"] = out
    fixed = json.dumps(d).encode()
    nc.to_json_bytes = lambda: fixed


_NC_CACHE = {}


def _build_moe_kernel(cap):
    """One expert's FFN over `cap` gathered tokens (SPMD on all 8 cores)."""
    if cap in _NC_CACHE:
        return _NC_CACHE[cap]

    bf16 = mybir.dt.bfloat16
    f32 = mybir.dt.float32
    Act = mybir.ActivationFunctionType

    nc = bass.Bass("TRN2", target_bir_lowering=False, debug=False,
                   num_devices=N_CORES)
    xT = nc.declare_dram_parameter("xT", [D_MODEL, cap], bf16, isOutput=False)
    w1t = nc.declare_dram_parameter("w1t", [D_MODEL, D_FF], bf16, isOutput=False)
    w2t = nc.declare_dram_parameter("w2t", [D_FF, D_MODEL], bf16, isOutput=False)
    b1 = nc.declare_dram_parameter("b1", [D_FF], f32, isOutput=False)
    b2 = nc.declare_dram_parameter("b2", [D_MODEL], f32, isOutput=False)
    wts = nc.declare_dram_parameter("wts", [P, cap], f32, isOutput=False)
    yT = nc.declare_dram_parameter("yT", [D_MODEL, cap], f32, isOutput=True)

    # DRAM views with 128-partition blocks pulled out
    xr = xT.ap().rearrange("(g p) t -> p g t", p=P)     # [128, 8, cap]
    w1r = w1t.ap().rearrange("(g p) f -> p g f", p=P)   # [128, 8, 4096]
    w2r = w2t.ap().rearrange("(g p) c -> p g c", p=P)   # [128, 32, 1024]
    b1r = b1.ap().rearrange("(g p) -> p g", p=P)        # [128, 32]
    b2r = b2.ap().rearrange("(g p) -> p g", p=P)        # [128, 8]
    yr = yT.ap().rearrange("(g p) t -> p g t", p=P)     # [128, 8, cap]

    # token tiles: full 512s plus an optional 128-multiple remainder
    tiles = []
    off = 0
    while off < cap:
        tw = min(TN, cap - off)
        tiles.append((off, tw))
        off += tw

    with tile.TileContext(nc) as tc:
        with (
            tc.tile_pool(name="weights", bufs=1) as wpool,
            tc.tile_pool(name="xin", bufs=3) as xpool,
            tc.tile_pool(name="wtp", bufs=3) as wtpool,
            tc.tile_pool(name="hbuf", bufs=1) as hpool,
            tc.tile_pool(name="yout", bufs=4) as ypool,
            tc.tile_pool(name="psum", bufs=4, space="PSUM") as psum,
        ):
            # DMA emission order is the SP ring's FIFO issue order; it is
            # chosen so PE can start ~8us in: tile-0 tokens first, then w1 in
            # M-strips (GEMM1 m-block 0 only needs strip 0 + xt0), then w2
            # (fully landed by the time GEMM2 of tile 0 starts at ~60us).
            # Later tiles' token loads queue behind the weights; they are
            # small and needed much later.
            x0 = xpool.tile([P, CB, TN], bf16, tag="xt")
            nc.sync.dma_start(x0[:, :, :tiles[0][1]], xr[:, :, ds(0, tiles[0][1])])

            MS = 512  # w1 M-strip width (4 m-blocks, ~1 MiB per DMA)
            w1_sb = wpool.tile([P, CB, D_FF], bf16, tag="w1")
            nc.sync.dma_start(w1_sb[:, :, 0:MS], w1r[:, :, 0:MS])

            # small/late-needed loads go behind the PE-critical first strip
            b1_sb = wpool.tile([P, FB], f32, tag="b1")
            nc.sync.dma_start(b1_sb[:], b1r)
            b2_sb = wpool.tile([P, CB], f32, tag="b2")
            nc.sync.dma_start(b2_sb[:], b2r)
            w0 = wtpool.tile([P, TN], f32, tag="wt")
            nc.sync.dma_start(w0[:, :tiles[0][1]], wts[:, ds(0, tiles[0][1])])

            for s in range(MS, D_FF, MS):
                nc.sync.dma_start(w1_sb[:, :, s:s + MS], w1r[:, :, s:s + MS])
            w2_sb = wpool.tile([P, FB, D_MODEL], bf16, tag="w2")
            for k in range(0, FB, 4):
                nc.sync.dma_start(w2_sb[:, k:k + 4, :], w2r[:, k:k + 4, :])

            for ti, (off, tw) in enumerate(tiles):
                if ti == 0:
                    xt, wt = x0, w0
                else:
                    xt = xpool.tile([P, CB, TN], bf16, tag="xt")
                    nc.sync.dma_start(xt[:, :, :tw], xr[:, :, ds(off, tw)])
                    wt = wtpool.tile([P, TN], f32, tag="wt")
                    nc.sync.dma_start(wt[:, :tw], wts[:, ds(off, tw)])

                ht = hpool.tile([P, FB, TN], bf16, tag="ht")
                # GEMM1 + gelu: hT[f,t] = gelu(sum_c w1T[c,f] * xT[c,t] + b1[f])
                for m in range(FB):
                    ph = psum.tile([P, TN], f32, tag="ph")
                    for k in range(CB):
                        nc.tensor.matmul(
                            ph[:, :tw],
                            lhsT=w1_sb[:, k, ts(m, P)],
                            rhs=xt[:, k, :tw],
                            start=(k == 0), stop=(k == CB - 1),
                        )
                    nc.scalar.activation(ht[:, m, :tw], ph[:, :tw], Act.Gelu,
                                         bias=b1_sb[:, m:m + 1])
                # GEMM2 + bias + combine-weight: yT[c,t]
                for c in range(CB):
                    py = psum.tile([P, TN], f32, tag="py")
                    for k in range(FB):
                        nc.tensor.matmul(
                            py[:, :tw],
                            lhsT=w2_sb[:, k, ts(c, P)],
                            rhs=ht[:, k, :tw],
                            start=(k == 0), stop=(k == FB - 1),
                        )
                    yt = ypool.tile([P, TN], f32, tag="yt")
                    nc.scalar.add(yt[:, :tw], py[:, :tw], b2_sb[:, c:c + 1])
                    nc.vector.tensor_mul(yt[:, :tw], yt[:, :tw], wt[:, :tw])
                    nc.sync.dma_start(yr[:, c, ds(off, tw)], yt[:, :tw])

    _fix_multiwait_bir(nc)
    _NC_CACHE[cap] = nc
    return nc


def _route(xf, router_w, k):
    """Replicate the reference router numerics (f32 softmax, top-k, renorm)."""
    logits = xf @ router_w.T.astype(np.float32)          # [T, E]
    m = logits.max(axis=-1, keepdims=True)
    e = np.exp(logits - m, dtype=np.float32)
    probs = e / e.sum(axis=-1, keepdims=True)
    # descending, ties -> lower index first (matches jax.lax.top_k)
    idx = np.argsort(-probs, axis=-1, kind="stable")[:, :k]   # [T, k]
    w = np.take_along_axis(probs, idx, axis=-1)               # [T, k]
    w = w / (w.sum(axis=-1, keepdims=True) + 1e-9)
    return idx, w


def kernel(x, router_w, expert_w1, expert_b1, expert_w2, expert_b2, top_k):
    x = np.asarray(x)
    router_w = np.asarray(router_w, dtype=np.float32)
    expert_w1 = np.asarray(expert_w1, dtype=np.float32)
    expert_b1 = np.asarray(expert_b1, dtype=np.float32)
    expert_w2 = np.asarray(expert_w2, dtype=np.float32)
    expert_b2 = np.asarray(expert_b2, dtype=np.float32)
    k = int(np.asarray(top_k))
    Bq, Nq, C = x.shape
    Tq = Bq * Nq
    E = expert_w1.shape[0]
    xf = np.ascontiguousarray(x.reshape(Tq, C), dtype=np.float32)

    idx, w = _route(xf, router_w, k)

    # per-expert token lists + combine weights
    tok_idx, tok_w = [], []
    for e in range(E):
        mask = idx == e                                   # [T, k]
        sel = np.nonzero(mask.any(axis=-1))[0]
        tok_idx.append(sel)
        tok_w.append((w * mask).sum(axis=-1)[sel].astype(np.float32))
    counts = [len(s) for s in tok_idx]
    cap = max(P, -(-max(counts) // P) * P)

    nc = _build_moe_kernel(cap)

    in_maps = []
    for e in range(E):
        cnt = counts[e]
        xT = np.zeros((C, cap), dtype=ml_dtypes.bfloat16)
        xT[:, :cnt] = xf[tok_idx[e]].T
        wtsP = np.zeros((P, cap), dtype=np.float32)
        wtsP[:, :cnt] = tok_w[e][None, :]
        in_maps.append({
            "xT": xT,
            "w1t": np.ascontiguousarray(expert_w1[e].T).astype(ml_dtypes.bfloat16),
            "w2t": np.ascontiguousarray(expert_w2[e].T).astype(ml_dtypes.bfloat16),
            "b1": np.ascontiguousarray(expert_b1[e]),
            "b2": np.ascontiguousarray(expert_b2[e]),
            "wts": wtsP,
        })

    trace = os.environ.get("BASS_MOE_TRACE") == "1"
    res = run_bass_kernel_spmd(
        nc, in_maps, core_ids=list(range(N_CORES)),
        trace=trace,
        tmpdir=os.environ.get("BASS_MOE_TMPDIR") if trace else None,
    )
    if trace:
        kernel.last_exec_time_ns = res.exec_time_ns
        kernel.last_trace = (res.instructions_and_trace or (None, None))[1]

    out = np.zeros((Tq, C), dtype=np.float32)
    for e in range(E):
        cnt = counts[e]
        if cnt:
            out[tok_idx[e]] += res.results[e]["yT"][:, :cnt].T
    return out.reshape(Bq, Nq, C).astype(x.dtype)


# revision 4
# speedup vs baseline: 1.0850x; 1.0393x over previous
"""MoE layer (top-k routing) on 8 Trainium2 NeuronCores.

Strategy (expert-parallel, per sharding hint):
  - Host: compute router softmax + top-k (0.1% of FLOPs), build per-expert
    token batches (the "all-to-all dispatch by expert assignment" is realized
    while constructing the per-core SPMD input maps).
  - Core e owns expert e: computes y = (gelu(x @ w1.T + b1) @ w2.T + b2) * w
    for its gathered tokens in bf16 (fp32 PSUM accumulation), with the
    combine weight multiplied in on-device.
  - Host: scatter-add the per-expert results back into the [B,N,C] output.

Data layout on device is "transposed activation" space so both GEMMs chain
without transposes: xT [C, T], hT [F, T], yT [C, T] with the contraction dim
on SBUF partitions. Weights are passed pre-transposed (w1T = w1.T [C,F],
w2T = w2.T [F,C]) and stay SBUF-resident for the whole kernel.
"""

import json
import os
import sys
import types

import numpy as np
import ml_dtypes

D_MODEL = 1024
D_FF = 4096
N_EXPERTS = 8
N_CORES = 8
B, N = 8, 2048
T = B * N

P = 128
CB = D_MODEL // P      # 8 c-blocks of 128
FB = D_FF // P         # 32 f-blocks of 128
TN = 512               # token tile (matmul moving free dim / one PSUM bank)


def _shim_axon_hooks():
    """Register the NTFF profile hook bass_utils looks for under axon.

    The image's `antenv` stub lacks `axon_hooks`; the ctypes machinery in
    trn_agent_boot provides the same hook. Only needed for trace runs, but
    harmless otherwise."""
    if "antenv.axon_hooks" in sys.modules:
        return
    try:
        import trn_agent_boot.trn_boot as _tb
        hook = _tb._ntff_profile_via_ctypes("/opt/axon/libaxon_pjrt.so")
    except Exception:
        hook = None
    mod = types.ModuleType("antenv.axon_hooks")
    mod.get_axon_ntff_profile_hook = lambda: hook
    mod.set_axon_ntff_profile_hook = lambda h: None
    sys.modules["antenv.axon_hooks"] = mod


_shim_axon_hooks()

import concourse.bass as bass            # noqa: E402
import concourse.tile as tile            # noqa: E402
from concourse import mybir              # noqa: E402
from concourse.bass import ds, ts        # noqa: E402
from concourse.bass_utils import run_bass_kernel_spmd  # noqa: E402


def _fix_multiwait_bir(nc):
    """Split instructions carrying >1 sync wait (the TileContext tail drain)
    into single-wait NoOps + the original instruction with one wait; this
    walrus build rejects multi-wait CTRL instructions."""
    raw = bass.Bass.to_json_bytes(nc)
    d = json.loads(raw)
    for f in d["functions"]:
        for b in f["blocks"]:
            out = []
            for i in b["instructions"]:
                si = i.get("sync_info") or {}
                waits = si.get("on_wait") or []
                if len(waits) > 1:
                    for k, w in enumerate(waits[:-1]):
                        out.append({
                            "name": f"{i['name']}_wsplit{k}",
                            "engine": i["engine"],
                            "ins": [], "outs": [],
                            "opcode": "NoOp",
                            "sync_info": {"on_update": [], "on_wait": [w]},
                        })
                    si["on_wait"] = [waits[-1]]
                out.append(i)
            b["instructions"] = out
    fixed = json.dumps(d).encode()
    nc.to_json_bytes = lambda: fixed


_NC_CACHE = {}


def _build_moe_kernel(cap):
    """One expert's FFN over `cap` gathered tokens (SPMD on all 8 cores)."""
    if cap in _NC_CACHE:
        return _NC_CACHE[cap]

    bf16 = mybir.dt.bfloat16
    f32 = mybir.dt.float32
    Act = mybir.ActivationFunctionType

    nc = bass.Bass("TRN2", target_bir_lowering=False, debug=False,
                   num_devices=N_CORES)
    xT = nc.declare_dram_parameter("xT", [D_MODEL, cap], bf16, isOutput=False)
    w1t = nc.declare_dram_parameter("w1t", [D_MODEL, D_FF], bf16, isOutput=False)
    w2t = nc.declare_dram_parameter("w2t", [D_FF, D_MODEL], bf16, isOutput=False)
    b1 = nc.declare_dram_parameter("b1", [D_FF], f32, isOutput=False)
    b2 = nc.declare_dram_parameter("b2", [D_MODEL], f32, isOutput=False)
    wts = nc.declare_dram_parameter("wts", [P, cap], f32, isOutput=False)
    yT = nc.declare_dram_parameter("yT", [D_MODEL, cap], f32, isOutput=True)

    # DRAM views with 128-partition blocks pulled out
    xr = xT.ap().rearrange("(g p) t -> p g t", p=P)     # [128, 8, cap]
    w1r = w1t.ap().rearrange("(g p) f -> p g f", p=P)   # [128, 8, 4096]
    w2r = w2t.ap().rearrange("(g p) c -> p g c", p=P)   # [128, 32, 1024]
    b1r = b1.ap().rearrange("(g p) -> p g", p=P)        # [128, 32]
    b2r = b2.ap().rearrange("(g p) -> p g", p=P)        # [128, 8]
    yr = yT.ap().rearrange("(g p) t -> p g t", p=P)     # [128, 8, cap]

    # token tiles: full 512s plus an optional 128-multiple remainder
    tiles = []
    off = 0
    while off < cap:
        tw = min(TN, cap - off)
        tiles.append((off, tw))
        off += tw

    with tile.TileContext(nc) as tc:
        with (
            tc.tile_pool(name="weights", bufs=1) as wpool,
            tc.tile_pool(name="xin", bufs=3) as xpool,
            tc.tile_pool(name="wtp", bufs=3) as wtpool,
            tc.tile_pool(name="hbuf", bufs=1) as hpool,
            tc.tile_pool(name="yout", bufs=4) as ypool,
            tc.tile_pool(name="psum", bufs=4, space="PSUM") as psum,
        ):
            # DMA emission order is the SP ring's FIFO issue order; it is
            # chosen so PE can start ~8us in: tile-0 tokens first, then w1 in
            # M-strips (GEMM1 m-block 0 only needs strip 0 + xt0), then w2
            # (fully landed by the time GEMM2 of tile 0 starts at ~60us).
            # Later tiles' token loads queue behind the weights; they are
            # small and needed much later.
            x0 = xpool.tile([P, CB, TN], bf16, tag="xt")
            nc.sync.dma_start(x0[:, :, :tiles[0][1]], xr[:, :, ds(0, tiles[0][1])])

            MS = 512  # w1 M-strip width (4 m-blocks, ~1 MiB per DMA)
            w1_sb = wpool.tile([P, CB, D_FF], bf16, tag="w1")
            nc.sync.dma_start(w1_sb[:, :, 0:MS], w1r[:, :, 0:MS])

            # small/late-needed loads go behind the PE-critical first strip
            b1_sb = wpool.tile([P, FB], f32, tag="b1")
            nc.sync.dma_start(b1_sb[:], b1r)
            b2_sb = wpool.tile([P, CB], f32, tag="b2")
            nc.sync.dma_start(b2_sb[:], b2r)
            w0 = wtpool.tile([P, TN], f32, tag="wt")
            nc.sync.dma_start(w0[:, :tiles[0][1]], wts[:, ds(0, tiles[0][1])])

            for s in range(MS, D_FF, MS):
                nc.sync.dma_start(w1_sb[:, :, s:s + MS], w1r[:, :, s:s + MS])
            w2_sb = wpool.tile([P, FB, D_MODEL], bf16, tag="w2")
            for k in range(0, FB, 4):
                nc.sync.dma_start(w2_sb[:, k:k + 4, :], w2r[:, k:k + 4, :])

            for ti, (off, tw) in enumerate(tiles):
                if ti == 0:
                    xt, wt = x0, w0
                else:
                    xt = xpool.tile([P, CB, TN], bf16, tag="xt")
                    nc.sync.dma_start(xt[:, :, :tw], xr[:, :, ds(off, tw)])
                    wt = wtpool.tile([P, TN], f32, tag="wt")
                    nc.sync.dma_start(wt[:, :tw], wts[:, ds(off, tw)])

                ht = hpool.tile([P, FB, TN], bf16, tag="ht")
                # GEMM1 + gelu: hT[f,t] = gelu(sum_c w1T[c,f] * xT[c,t] + b1[f])
                for m in range(FB):
                    ph = psum.tile([P, TN], f32, tag="ph")
                    for k in range(CB):
                        nc.tensor.matmul(
                            ph[:, :tw],
                            lhsT=w1_sb[:, k, ts(m, P)],
                            rhs=xt[:, k, :tw],
                            start=(k == 0), stop=(k == CB - 1),
                        )
                    nc.scalar.activation(ht[:, m, :tw], ph[:, :tw], Act.Gelu,
                                         bias=b1_sb[:, m:m + 1])
                # GEMM2 + bias + combine-weight: yT[c,t]
                for c in range(CB):
                    py = psum.tile([P, TN], f32, tag="py")
                    for k in range(FB):
                        nc.tensor.matmul(
                            py[:, :tw],
                            lhsT=w2_sb[:, k, ts(c, P)],
                            rhs=ht[:, k, :tw],
                            start=(k == 0), stop=(k == FB - 1),
                        )
                    yt = ypool.tile([P, TN], f32, tag="yt")
                    nc.scalar.add(yt[:, :tw], py[:, :tw], b2_sb[:, c:c + 1])
                    nc.vector.tensor_mul(yt[:, :tw], yt[:, :tw], wt[:, :tw])
                    nc.sync.dma_start(yr[:, c, ds(off, tw)], yt[:, :tw])

    _fix_multiwait_bir(nc)
    _NC_CACHE[cap] = nc
    return nc


def _route(xf, router_w, k):
    """Replicate the reference router numerics (f32 softmax, top-k, renorm)."""
    logits = xf @ router_w.T.astype(np.float32)          # [T, E]
    m = logits.max(axis=-1, keepdims=True)
    e = np.exp(logits - m, dtype=np.float32)
    probs = e / e.sum(axis=-1, keepdims=True)
    # descending, ties -> lower index first (matches jax.lax.top_k)
    idx = np.argsort(-probs, axis=-1, kind="stable")[:, :k]   # [T, k]
    w = np.take_along_axis(probs, idx, axis=-1)               # [T, k]
    w = w / (w.sum(axis=-1, keepdims=True) + 1e-9)
    return idx, w


def kernel(x, router_w, expert_w1, expert_b1, expert_w2, expert_b2, top_k):
    x = np.asarray(x)
    router_w = np.asarray(router_w, dtype=np.float32)
    expert_w1 = np.asarray(expert_w1, dtype=np.float32)
    expert_b1 = np.asarray(expert_b1, dtype=np.float32)
    expert_w2 = np.asarray(expert_w2, dtype=np.float32)
    expert_b2 = np.asarray(expert_b2, dtype=np.float32)
    k = int(np.asarray(top_k))
    Bq, Nq, C = x.shape
    Tq = Bq * Nq
    E = expert_w1.shape[0]
    xf = np.ascontiguousarray(x.reshape(Tq, C), dtype=np.float32)

    idx, w = _route(xf, router_w, k)

    # per-expert token lists + combine weights
    tok_idx, tok_w = [], []
    for e in range(E):
        mask = idx == e                                   # [T, k]
        sel = np.nonzero(mask.any(axis=-1))[0]
        tok_idx.append(sel)
        tok_w.append((w * mask).sum(axis=-1)[sel].astype(np.float32))
    counts = [len(s) for s in tok_idx]
    # token dim is a matmul free dim — only needs 16-alignment for DMA
    # efficiency, not 128 (partition alignment applies to C/F, not tokens)
    cap = max(P, -(-max(counts) // 16) * 16)

    nc = _build_moe_kernel(cap)

    in_maps = []
    for e in range(E):
        cnt = counts[e]
        xT = np.zeros((C, cap), dtype=ml_dtypes.bfloat16)
        xT[:, :cnt] = xf[tok_idx[e]].T
        wtsP = np.zeros((P, cap), dtype=np.float32)
        wtsP[:, :cnt] = tok_w[e][None, :]
        in_maps.append({
            "xT": xT,
            "w1t": np.ascontiguousarray(expert_w1[e].T).astype(ml_dtypes.bfloat16),
            "w2t": np.ascontiguousarray(expert_w2[e].T).astype(ml_dtypes.bfloat16),
            "b1": np.ascontiguousarray(expert_b1[e]),
            "b2": np.ascontiguousarray(expert_b2[e]),
            "wts": wtsP,
        })

    trace = os.environ.get("BASS_MOE_TRACE") == "1"
    res = run_bass_kernel_spmd(
        nc, in_maps, core_ids=list(range(N_CORES)),
        trace=trace,
        tmpdir=os.environ.get("BASS_MOE_TMPDIR") if trace else None,
    )
    if trace:
        kernel.last_exec_time_ns = res.exec_time_ns
        kernel.last_trace = (res.instructions_and_trace or (None, None))[1]

    out = np.zeros((Tq, C), dtype=np.float32)
    for e in range(E):
        cnt = counts[e]
        if cnt:
            out[tok_idx[e]] += res.results[e]["yT"][:, :cnt].T
    return out.reshape(Bq, Nq, C).astype(x.dtype)


# revision 5
# speedup vs baseline: 1.0854x; 1.0003x over previous
"""MoE layer (top-k routing) on 8 Trainium2 NeuronCores.

Expert-parallel per the sharding hint: the host computes router softmax +
top-k (0.1% of FLOPs) and realizes the "all-to-all dispatch by expert
assignment" while building the per-core SPMD input maps; each core runs the
expert FFN in bf16 (fp32 PSUM accumulation) with the combine weight
multiplied in on-device; the host scatter-adds results back to [B,N,C].

For load balance, each expert's FFN is split along the D_FF axis into two
half-units (exact: gelu is elementwise over F,
and GEMM2 contracts F, so y = y_half0 + y_half1). The 16 half-units are
assigned two per core: the 4 largest experts' halves fill slot class A, the
4 smallest fill slot class B. The SPMD program pads slot A to the largest
top-4 count and slot B to the largest bottom-4 count, so per-core padded
work drops from 2*max(counts) to max(top4)+max(bottom4). Host sums the two
half partials per expert during the scatter-add.
"""

import json
import os
import sys
import types

import numpy as np
import ml_dtypes

D_MODEL = 1024
D_FF = 4096
N_EXPERTS = 8
N_CORES = 8

P = 128
CB = D_MODEL // P      # 8 c-blocks of 128
FH = D_FF // 2         # F half = 2048
FBH = FH // P          # 16 f-blocks per half
TN = 512               # token tile (matmul moving free dim / one PSUM bank)


def _shim_axon_hooks():
    if "antenv.axon_hooks" in sys.modules:
        return
    try:
        import trn_agent_boot.trn_boot as _tb
        hook = _tb._ntff_profile_via_ctypes("/opt/axon/libaxon_pjrt.so")
    except Exception:
        hook = None
    mod = types.ModuleType("antenv.axon_hooks")
    mod.get_axon_ntff_profile_hook = lambda: hook
    mod.set_axon_ntff_profile_hook = lambda h: None
    sys.modules["antenv.axon_hooks"] = mod


_shim_axon_hooks()

import concourse.bass as bass            # noqa: E402
import concourse.tile as tile            # noqa: E402
from concourse import mybir              # noqa: E402
from concourse.bass import ds, ts        # noqa: E402
from concourse.bass_utils import run_bass_kernel_spmd  # noqa: E402


def _fix_multiwait_bir(nc):
    """Split instructions carrying >1 sync wait (the TileContext tail drain)
    into single-wait NoOps; this walrus build rejects multi-wait CTRL
    instructions."""
    raw = bass.Bass.to_json_bytes(nc)
    d = json.loads(raw)
    for f in d["functions"]:
        for b in f["blocks"]:
            out = []
            for i in b["instructions"]:
                si = i.get("sync_info") or {}
                waits = si.get("on_wait") or []
                if len(waits) > 1:
                    for k, w in enumerate(waits[:-1]):
                        out.append({
                            "name": f"{i['name']}_wsplit{k}",
                            "engine": i["engine"],
                            "ins": [], "outs": [],
                            "opcode": "NoOp",
                            "sync_info": {"on_update": [], "on_wait": [w]},
                        })
                    si["on_wait"] = [waits[-1]]
                out.append(i)
            b["instructions"] = out
    fixed = json.dumps(d).encode()
    nc.to_json_bytes = lambda: fixed


_NC_CACHE = {}


def _token_tiles(cap):
    tiles, off = [], 0
    while off < cap:
        tw = min(TN, cap - off)
        tiles.append((off, tw))
        off += tw
    return tiles


def _build_moe_kernel(cap_a, cap_b):
    """Two half-expert FFN units per core (slot A then slot B), SPMD x8."""
    key = (cap_a, cap_b)
    if key in _NC_CACHE:
        return _NC_CACHE[key]

    bf16 = mybir.dt.bfloat16
    f32 = mybir.dt.float32
    Act = mybir.ActivationFunctionType

    nc = bass.Bass("TRN2", target_bir_lowering=False, debug=False,
                   num_devices=N_CORES)

    units = []
    for slot, cap in (("A", cap_a), ("B", cap_b)):
        u = {"cap": cap, "slot": slot}
        u["xT"] = nc.declare_dram_parameter(f"xT{slot}", [D_MODEL, cap], bf16, isOutput=False)
        u["w1t"] = nc.declare_dram_parameter(f"w1t{slot}", [D_MODEL, FH], bf16, isOutput=False)
        u["w2t"] = nc.declare_dram_parameter(f"w2t{slot}", [FH, D_MODEL], bf16, isOutput=False)
        u["b1"] = nc.declare_dram_parameter(f"b1{slot}", [FH], f32, isOutput=False)
        u["b2"] = nc.declare_dram_parameter(f"b2{slot}", [D_MODEL], f32, isOutput=False)
        u["wts"] = nc.declare_dram_parameter(f"wts{slot}", [P, cap], f32, isOutput=False)
        u["yT"] = nc.declare_dram_parameter(f"yT{slot}", [D_MODEL, cap], f32, isOutput=True)
        u["xr"] = u["xT"].ap().rearrange("(g p) t -> p g t", p=P)
        u["w1r"] = u["w1t"].ap().rearrange("(g p) f -> p g f", p=P)   # [128, 8, 2048]
        u["w2r"] = u["w2t"].ap().rearrange("(g p) c -> p g c", p=P)   # [128, 16, 1024]
        u["b1r"] = u["b1"].ap().rearrange("(g p) -> p g", p=P)        # [128, 16]
        u["b2r"] = u["b2"].ap().rearrange("(g p) -> p g", p=P)        # [128, 8]
        u["yr"] = u["yT"].ap().rearrange("(g p) t -> p g t", p=P)
        u["tiles"] = _token_tiles(cap)
        units.append(u)

    MS = 512  # w1 M-strip width (~1 MiB per DMA)

    with tile.TileContext(nc) as tc:
        with (
            tc.tile_pool(name="weights", bufs=1) as wpool,
            tc.tile_pool(name="xin", bufs=3) as xpool,
            tc.tile_pool(name="wtp", bufs=3) as wtpool,
            tc.tile_pool(name="hbuf", bufs=1) as hpool,
            tc.tile_pool(name="yout", bufs=2) as ypool,
            tc.tile_pool(name="psum", bufs=4, space="PSUM") as psum,
        ):
            # ---- loads: unit A's critical path first, then the rest ----
            ua, ub = units
            ua["x0"] = xpool.tile([P, CB, TN], bf16, tag="xt", name="x0A")
            nc.sync.dma_start(ua["x0"][:, :, :ua["tiles"][0][1]],
                              ua["xr"][:, :, ds(0, ua["tiles"][0][1])])
            # first w1 strip rides the ACT HWDGE ring so it overlaps x0A's
            # load on the SP ring — shortens the PE-start critical path
            ua["w1_sb"] = wpool.tile([P, CB, FH], bf16, tag="w1A", name="w1A")
            nc.scalar.dma_start(ua["w1_sb"][:, :, 0:MS], ua["w1r"][:, :, 0:MS])

            for u in units:
                u["b1_sb"] = wpool.tile([P, FBH], f32, tag=f"b1{u['slot']}", name=f"b1{u['slot']}")
                nc.sync.dma_start(u["b1_sb"][:], u["b1r"])
                u["b2_sb"] = wpool.tile([P, CB], f32, tag=f"b2{u['slot']}", name=f"b2{u['slot']}")
                nc.sync.dma_start(u["b2_sb"][:], u["b2r"])
            ua["wt0"] = wtpool.tile([P, TN], f32, tag="wt", name="wt0A")
            nc.sync.dma_start(ua["wt0"][:, :ua["tiles"][0][1]],
                              ua["wts"][:, ds(0, ua["tiles"][0][1])])

            for s in range(MS, FH, MS):
                nc.sync.dma_start(ua["w1_sb"][:, :, s:s + MS], ua["w1r"][:, :, s:s + MS])
            ua["w2_sb"] = wpool.tile([P, FBH, D_MODEL], bf16, tag="w2A", name="w2A")
            for k in range(0, FBH, 4):
                nc.sync.dma_start(ua["w2_sb"][:, k:k + 4, :], ua["w2r"][:, k:k + 4, :])

            ub["w1_sb"] = wpool.tile([P, CB, FH], bf16, tag="w1B", name="w1B")
            for s in range(0, FH, MS):
                nc.sync.dma_start(ub["w1_sb"][:, :, s:s + MS], ub["w1r"][:, :, s:s + MS])
            ub["w2_sb"] = wpool.tile([P, FBH, D_MODEL], bf16, tag="w2B", name="w2B")
            for k in range(0, FBH, 4):
                nc.sync.dma_start(ub["w2_sb"][:, k:k + 4, :], ub["w2r"][:, k:k + 4, :])

            # ---- compute: unit A tiles, then unit B tiles ----
            for u in units:
                for ti, (off, tw) in enumerate(u["tiles"]):
                    if ti == 0 and "x0" in u:
                        xt, wt = u["x0"], u["wt0"]
                    else:
                        xt = xpool.tile([P, CB, TN], bf16, tag="xt")
                        nc.sync.dma_start(xt[:, :, :tw], u["xr"][:, :, ds(off, tw)])
                        wt = wtpool.tile([P, TN], f32, tag="wt")
                        nc.sync.dma_start(wt[:, :tw], u["wts"][:, ds(off, tw)])

                    ht = hpool.tile([P, FBH, TN], bf16, tag="ht")
                    for m in range(FBH):
                        ph = psum.tile([P, TN], f32, tag="ph")
                        for k in range(CB):
                            nc.tensor.matmul(
                                ph[:, :tw],
                                lhsT=u["w1_sb"][:, k, ts(m, P)],
                                rhs=xt[:, k, :tw],
                                start=(k == 0), stop=(k == CB - 1),
                            )
                        nc.scalar.activation(ht[:, m, :tw], ph[:, :tw], Act.Gelu,
                                             bias=u["b1_sb"][:, m:m + 1])
                    last = (u is ub) and (ti == len(u["tiles"]) - 1)
                    yt = ypool.tile([P, CB, TN], f32, tag="yt")
                    for c in range(CB):
                        py = psum.tile([P, TN], f32, tag="py")
                        for k in range(FBH):
                            nc.tensor.matmul(
                                py[:, :tw],
                                lhsT=u["w2_sb"][:, k, ts(c, P)],
                                rhs=ht[:, k, :tw],
                                start=(k == 0), stop=(k == FBH - 1),
                            )
                        nc.scalar.add(yt[:, c, :tw], py[:, :tw], u["b2_sb"][:, c:c + 1])
                        nc.vector.tensor_mul(yt[:, c, :tw], yt[:, c, :tw], wt[:, :tw])
                        if last:
                            # final tile: per-block stores overlap the tail
                            # GEMM2 blocks instead of one post-loop DMA
                            nc.sync.dma_start(u["yr"][:, c, ds(off, tw)],
                                              yt[:, c, :tw])
                    if not last:
                        nc.sync.dma_start(u["yr"][:, :, ds(off, tw)], yt[:, :, :tw])

    _fix_multiwait_bir(nc)
    _NC_CACHE[key] = nc
    return nc


def _route(xf, router_w, k):
    """Replicate the reference router numerics (f32 softmax, top-k, renorm)."""
    logits = xf @ router_w.T.astype(np.float32)          # [T, E]
    m = logits.max(axis=-1, keepdims=True)
    e = np.exp(logits - m, dtype=np.float32)
    probs = e / e.sum(axis=-1, keepdims=True)
    idx = np.argsort(-probs, axis=-1, kind="stable")[:, :k]   # [T, k]
    w = np.take_along_axis(probs, idx, axis=-1)               # [T, k]
    w = w / (w.sum(axis=-1, keepdims=True) + 1e-9)
    return idx, w


def _align16(n):
    return max(P, -(-n // 16) * 16)


def kernel(x, router_w, expert_w1, expert_b1, expert_w2, expert_b2, top_k):
    x = np.asarray(x)
    router_w = np.asarray(router_w, dtype=np.float32)
    expert_w1 = np.asarray(expert_w1, dtype=np.float32)
    expert_b1 = np.asarray(expert_b1, dtype=np.float32)
    expert_w2 = np.asarray(expert_w2, dtype=np.float32)
    expert_b2 = np.asarray(expert_b2, dtype=np.float32)
    k = int(np.asarray(top_k))
    Bq, Nq, C = x.shape
    Tq = Bq * Nq
    E = expert_w1.shape[0]
    xf = np.ascontiguousarray(x.reshape(Tq, C), dtype=np.float32)

    idx, w = _route(xf, router_w, k)

    tok_idx, tok_w = [], []
    for e in range(E):
        mask = idx == e
        sel = np.nonzero(mask.any(axis=-1))[0]
        tok_idx.append(sel)
        tok_w.append((w * mask).sum(axis=-1)[sel].astype(np.float32))
    counts = np.array([len(s) for s in tok_idx])

    # slot A <- both halves of the 4 largest experts; slot B <- 4 smallest.
    order = np.argsort(-counts, kind="stable")
    big, small = order[:4], order[4:]
    cap_a = _align16(int(counts[big].max()))
    cap_b = _align16(int(counts[small].max()) if len(small) else P)

    nc = _build_moe_kernel(cap_a, cap_b)

    def unit_inputs(e, half, cap, slot):
        cnt = counts[e]
        f0, f1 = half * FH, (half + 1) * FH
        xT = np.zeros((C, cap), dtype=ml_dtypes.bfloat16)
        xT[:, :cnt] = xf[tok_idx[e]].T
        wtsP = np.zeros((P, cap), dtype=np.float32)
        wtsP[:, :cnt] = tok_w[e][None, :]
        b2 = expert_b2[e] if half == 0 else np.zeros(C, dtype=np.float32)
        return {
            f"xT{slot}": xT,
            f"w1t{slot}": np.ascontiguousarray(expert_w1[e, f0:f1].T).astype(ml_dtypes.bfloat16),
            f"w2t{slot}": np.ascontiguousarray(expert_w2[e, :, f0:f1].T).astype(ml_dtypes.bfloat16),
            f"b1{slot}": np.ascontiguousarray(expert_b1[e, f0:f1]),
            f"b2{slot}": np.ascontiguousarray(b2),
            f"wts{slot}": wtsP,
        }

    # core 2i / 2i+1 take halves 0/1 of big[i] in slot A and of small[i] in B
    assign = []   # per core: ((eA, halfA), (eB, halfB))
    for i in range(4):
        for h in range(2):
            assign.append(((int(big[i]), h), (int(small[i]), h)))

    in_maps = []
    for (ea, ha), (eb, hb) in assign:
        m = unit_inputs(ea, ha, cap_a, "A")
        m.update(unit_inputs(eb, hb, cap_b, "B"))
        in_maps.append(m)

    trace = os.environ.get("BASS_MOE_TRACE") == "1"
    res = run_bass_kernel_spmd(
        nc, in_maps, core_ids=list(range(N_CORES)),
        trace=trace,
        tmpdir=os.environ.get("BASS_MOE_TMPDIR") if trace else None,
    )
    if trace:
        kernel.last_exec_time_ns = res.exec_time_ns
        kernel.last_trace = (res.instructions_and_trace or (None, None))[1]

    out = np.zeros((Tq, C), dtype=np.float32)
    for core, ((ea, _), (eb, _)) in enumerate(assign):
        if counts[ea]:
            out[tok_idx[ea]] += res.results[core]["yTA"][:, :counts[ea]].T
        if counts[eb]:
            out[tok_idx[eb]] += res.results[core]["yTB"][:, :counts[eb]].T
    return out.reshape(Bq, Nq, C).astype(x.dtype)
